# revision 13
# baseline (speedup 1.0000x reference)
"""FlowNetC correlation layer on 8 Trainium2 NeuronCores.

Math: out[b, d, y, x] = (1/256) * sum_c in1[b,c,y,x] * in2pad[b,c,y+dy,x+dx]
with (dy, dx) on a 21x21 stride-2 grid spanning [-20, 20], zero padding 20.

Strategy (per core = one batch sample; batch is exactly 8):
- Displacements have stride 2, so the problem splits into 4 independent parity
  classes. Each class: in1c [256, 32, 48] against a padded in2c [256, 52, 68]
  with stride-1 displacements dy', dx' in [0, 20].
- Gram band matmuls: per class and group of 4 subsampled x-columns, 4
  col-tiled matmuls (M=32 each, tile_position=(0, 32*xg)). Stationary is
  in1c[:, :, x0] (32 ys); moving is the 21-wide window in2c[:, :, x0:x0+21]
  over all 52 rows (N = 1092 split 504/504/84 across 3 PSUM banks). PSUM
  partition 32*xg + ys holds the 441-displacement window contiguously at
  columns [21*ys, 21*ys + 441).
- Evictions psum->band alternate DVE/ACT (both run in parallel).
- De-shear is ONE DMA per class: the diagonal access-pattern stride
  (FB + 21) advances one partition AND 21 elements, encoding the per-ys
  shear; 882-byte descriptor runs.
- TensorE transposes flip dense [pixel, d] tiles to [d, pixel]; scatter
  copies (DVE/ACT alternating) assemble a d-major bf16 raster; 4 output DMAs
  (gpsimd, bf16->f32 cast) write [441, 64, 96] with 24 KB runs per d.
- Matmul inputs are bf16; the 1/256 normalization is folded into in1's bf16
  cast exactly (exponent shift).
"""

import os
import sys

for _p in ("/opt/trn_rl_repo", "/root/.axon_site/_ro/trn_rl_repo"):
    if os.path.isdir(_p) and _p not in sys.path:
        sys.path.insert(0, _p)

from contextlib import ExitStack

import ml_dtypes
import numpy as np

import concourse.bacc as bacc
import concourse.bass as bass
import concourse.mybir as mybir
import concourse.tile as tile
from concourse.bass_utils import run_bass_kernel_spmd
from concourse.masks import make_identity

B, C, H, W = 8, 256, 64, 96
NYS, NXS = 32, 48          # subsampled class grid
RB, CB = 52, 68            # padded class grid (rows/cols)
ND = 441                   # displacements
WB = 1092                  # band width per xs-column (52 rows * 21 dx)
NG = 12                    # xs-column groups per class band
FB = NG * WB               # class band free size (13104)
DP = NG * ND               # dense free size (5292)
NPIX = H * W               # 6144
DCHUNKS = [(0, 128), (128, 128), (256, 128), (384, 57)]
GRAM_CHUNKS = [(0, 24), (24, 48), (48, 52)]  # ysB row ranges per PSUM bank

F32 = mybir.dt.float32
BF16 = mybir.dt.bfloat16


def build(reps=1):
    nc = bacc.Bacc("TRN2", target_bir_lowering=False, debug=False, num_devices=8)
    in1p = nc.declare_dram_parameter("in1p", [2, 128, 4, NXS, NYS], BF16, isOutput=False)
    in2p = nc.declare_dram_parameter("in2p", [4, 128, 2, RB, CB], BF16, isOutput=False)
    outp = nc.declare_dram_parameter("out", [ND, H, W], F32, isOutput=True)

    with tile.TileContext(nc) as tc:
        with ExitStack() as ctx:
            const_pool = ctx.enter_context(tc.tile_pool(name="const", bufs=1))
            in2_pool = ctx.enter_context(tc.tile_pool(name="in2", bufs=2))
            band_pool = ctx.enter_context(tc.tile_pool(name="band", bufs=2))
            dense_pool = ctx.enter_context(tc.tile_pool(name="dense", bufs=2))
            out_pool = ctx.enter_context(tc.tile_pool(name="outsb", bufs=1))
            hbm_pool = ctx.enter_context(tc.tile_pool(name="hbm", bufs=2, space="DRAM"))
            pg_pool = ctx.enter_context(tc.tile_pool(name="pg", bufs=2, space="PSUM"))
            pt_pool = ctx.enter_context(tc.tile_pool(name="pt", bufs=2, space="PSUM"))

            ident = const_pool.tile([128, 128], BF16)
            make_identity(nc, ident)

            # resident in1: [c, k, cls, xs, ys]
            in1_sb = const_pool.tile([128, 2, 4, NXS, NYS], BF16)
            nc.sync.dma_start(
                out=bass.AP(in1_sb.tensor, in1_sb.offset,
                            [[2 * 4 * NXS * NYS, 128], [4 * NXS * NYS, 2], [1, 4 * NXS * NYS]]),
                in_=bass.AP(in1p, 0,
                            [[4 * NXS * NYS, 128], [128 * 4 * NXS * NYS, 2], [1, 4 * NXS * NYS]]),
            )

            # persistent d-major assembly buffers, one per d-chunk
            out_sb = [out_pool.tile([128, NPIX], BF16, tag=f"out{dc}", name=f"out_sb{dc}")
                      for dc in range(4)]

            def transpose_scatter(cid, dense, eng_flip):
                """PE transposes dense [pixel, d] to [d, pixel]; DVE/ACT
                scatter into the d-major raster assembly buffers."""
                py, px = cid // 2, cid % 2
                for dc, (d0, dcw) in enumerate(DCHUNKS):
                    for s in range(4):
                        pt = pt_pool.tile([128, 384], BF16)
                        for j in range(3):
                            nc.tensor.transpose(
                                pt[0:dcw, j * 128:(j + 1) * 128],
                                dense[:, 3 * s + j, d0:d0 + dcw],
                                ident[:],
                            )
                        ob = out_sb[dc]
                        src = bass.AP(pt.tensor, pt.offset,
                                      [[384, dcw], [128, 3], [32, 4], [1, 32]])
                        doff = 96 * py + px + 8 * (3 * s)
                        dst = bass.AP(ob.tensor, ob.offset + doff,
                                      [[NPIX, dcw], [8, 3], [2, 4], [192, 32]])
                        if eng_flip % 2 == 0:
                            nc.vector.tensor_copy(out=dst, in_=src)
                        else:
                            nc.scalar.copy(out=dst, in_=src)
                        eng_flip += 1
                return eng_flip

            eng_flip = 0
            pending = None  # (cid, dense) whose transpose stage is deferred
            for rep in range(reps):
              for cid in range(4):
                # one contiguous-per-partition load: [c, k, row, col]
                in2_sb = in2_pool.tile([128, 2, RB, CB], BF16)
                nc.scalar.dma_start(
                    out=bass.AP(in2_sb.tensor, in2_sb.offset,
                                [[2 * RB * CB, 128], [1, 2 * RB * CB]]),
                    in_=bass.AP(in2p, cid * 128 * 2 * RB * CB,
                                [[2 * RB * CB, 128], [1, 2 * RB * CB]]),
                )
                band = band_pool.tile([128, FB], BF16)
                evs = []
                for xsg in range(12):
                    pg = pg_pool.tile([128, 3, 512], F32)
                    for k in range(2):
                        for xg in range(4):
                            x0 = 4 * xsg + xg
                            lhsT = in1_sb[:, k, cid, x0, :]
                            for ch, (r0, r1) in enumerate(GRAM_CHUNKS):
                                ncols = (r1 - r0) * 21
                                rhs = in2_sb[:, k, r0:r1, x0:x0 + 21]
                                nc.tensor.matmul(
                                    pg[32 * xg:32 * (xg + 1), ch, 0:ncols],
                                    lhsT, rhs,
                                    start=(k == 0), stop=(k == 1),
                                    tile_position=(0, 32 * xg),
                                    skip_group_check=True,
                                )
                    # evict psum band into packed band columns; big chunk
                    # (banks 0-1, 1008 cols) and small chunk (bank 2, 84 cols)
                    # on opposite engines, alternating per xsg for balance.
                    big_src = bass.AP(pg.tensor, pg.offset, [[1536, 128], [512, 2], [1, 504]])
                    big_dst = bass.AP(band.tensor, band.offset + xsg * WB,
                                      [[FB, 128], [504, 2], [1, 504]])
                    small_src = pg[:, 2, 0:84]
                    small_dst = band[:, xsg * WB + 1008: xsg * WB + 1092]
                    if xsg % 2 == 0:
                        evs.append(nc.vector.tensor_copy(out=big_dst, in_=big_src))
                        evs.append(nc.scalar.copy(out=small_dst, in_=small_src))
                    else:
                        evs.append(nc.scalar.copy(out=big_dst, in_=big_src))
                        evs.append(nc.vector.tensor_copy(out=small_dst, in_=small_src))

                # de-shear via an HBM bounce: the write is a linear dump
                # (partition-dim first so the cost model prices it right);
                # the 4 per-xg read-backs absorb the 21*ys shear on the HBM
                # side, where strides are unconstrained.
                hb = hbm_pool.tile([128, FB], BF16)
                wr = nc.sync.dma_start(
                    out=bass.AP(hb.tensor, hb.offset, [[FB, 128], [1, FB]]),
                    in_=bass.AP(band.tensor, band.offset, [[FB, 128], [1, FB]]),
                )
                for ev in evs:
                    tile.add_dep_helper(wr.ins, ev.ins,
                                        reason="hb write needs all evictions")
                dense = dense_pool.tile([128, NG, ND], BF16)
                for xg in range(4):
                    src = bass.AP(hb.tensor, hb.offset + 32 * xg * FB,
                                  [[FB + 21, 32], [WB, NG], [1, ND]])
                    dst = bass.AP(dense.tensor, dense.offset + 32 * xg * DP,
                                  [[DP, 32], [ND, NG], [1, ND]])
                    eng = nc.scalar if xg % 2 == 0 else nc.sync
                    eng.dma_start(out=dst, in_=src)

                # transpose/scatter the PREVIOUS class so PE's in-order
                # stream never stalls on this class's bounce DMAs.
                if pending is not None:
                    eng_flip = transpose_scatter(*pending, eng_flip)
                pending = (cid, dense)

              eng_flip = transpose_scatter(*pending, eng_flip)
              pending = None

              # output: one cast DMA per d-chunk, 24KB contiguous runs per d
              for dc, (d0, dcw) in enumerate(DCHUNKS):
                  ob = out_sb[dc]
                  nc.gpsimd.dma_start(
                      out=bass.AP(outp, d0 * NPIX, [[NPIX, dcw], [1, NPIX]]),
                      in_=bass.AP(ob.tensor, ob.offset, [[NPIX, dcw], [1, NPIX]]),
                  )

    nc.compile()
    return nc


def prep_inputs(input1, input2):
    """Host-side: parity split, pad, bf16 cast, fold 1/256 into in1."""
    in_maps = []
    for b in range(B):
        a1 = (input1[b].astype(np.float32) / 256.0).reshape(2, 128, H, W)
        a2 = input2[b].astype(np.float32).reshape(2, 128, H, W)
        in1p = np.empty((2, 128, 4, NXS, NYS), dtype=ml_dtypes.bfloat16)
        in2p = np.zeros((4, 128, 2, RB, CB), dtype=ml_dtypes.bfloat16)
        for cid in range(4):
            py, px = cid // 2, cid % 2
            in1p[:, :, cid] = a1[:, :, py::2, px::2].transpose(0, 1, 3, 2).astype(ml_dtypes.bfloat16)
            in2p[cid, :, :, 10:42, 10:58] = a2[:, :, py::2, px::2].transpose(1, 0, 2, 3).astype(ml_dtypes.bfloat16)
        in_maps.append({"in1p": in1p, "in2p": in2p})
    return in_maps


_NC = None


def get_nc():
    global _NC
    if _NC is None:
        _NC = build()
    return _NC


def kernel(input1, input2):
    nc = get_nc()
    in_maps = prep_inputs(np.asarray(input1), np.asarray(input2))
    r = run_bass_kernel_spmd(nc, in_maps, core_ids=list(range(8)))
    return np.stack([r.results[i]["out"] for i in range(B)]).astype(np.float32)


# revision 17
# speedup vs baseline: 2.6811x; 2.6811x over previous
"""FlowNetC correlation layer on 8 Trainium2 NeuronCores.

Math: out[b, d, y, x] = (1/256) * sum_c in1[b,c,y,x] * in2pad[b,c,y+dy,x+dx]
with (dy, dx) on a 21x21 stride-2 grid spanning [-20, 20], zero padding 20.

Strategy (per core = one batch sample; batch is exactly 8):
- Displacements have stride 2, so the problem splits into 4 independent parity
  classes. Each class: in1c [256, 32, 48] against a padded in2c [256, 52, 68]
  with stride-1 displacements dy', dx' in [0, 20].
- Gram band matmuls: per class and group of 4 subsampled x-columns, 4
  col-tiled matmuls (M=32 each, tile_position=(0, 32*xg)). Stationary is
  in1c[:, :, x0] (32 ys); moving is the 21-wide window in2c[:, :, x0:x0+21]
  over all 52 rows (N = 1092 split 504/504/84 across 3 PSUM banks). PSUM
  partition 32*xg + ys holds the 441-displacement window contiguously at
  columns [21*ys, 21*ys + 441).
- Evictions psum->band alternate DVE/ACT (both run in parallel).
- De-shear is ONE DMA per class: the diagonal access-pattern stride
  (FB + 21) advances one partition AND 21 elements, encoding the per-ys
  shear; 882-byte descriptor runs.
- TensorE transposes flip dense [pixel, d] tiles to [d, pixel]; scatter
  copies (DVE/ACT alternating) assemble a d-major bf16 raster; 4 output DMAs
  (gpsimd, bf16->f32 cast) write [441, 64, 96] with 24 KB runs per d.
- Matmul inputs are bf16; the 1/256 normalization is folded into in1's bf16
  cast exactly (exponent shift).
"""

import os
import sys

for _p in ("/opt/trn_rl_repo", "/root/.axon_site/_ro/trn_rl_repo"):
    if os.path.isdir(_p) and _p not in sys.path:
        sys.path.insert(0, _p)

from contextlib import ExitStack

import ml_dtypes
import numpy as np

import concourse.bacc as bacc
import concourse.bass as bass
import concourse.mybir as mybir
import concourse.tile as tile
from concourse.bass_utils import run_bass_kernel_spmd
from concourse.masks import make_identity

B, C, H, W = 8, 256, 64, 96
NYS, NXS = 32, 48          # subsampled class grid
RB, CB = 52, 68            # padded class grid (rows/cols)
ND = 441                   # displacements
WB = 1092                  # band width per xs-column (52 rows * 21 dx)
NG = 12                    # xs-column groups per class band
FB = NG * WB               # class band free size (13104)
DP = NG * ND               # dense free size (5292)
NPIX = H * W               # 6144
DCHUNKS = [(0, 128), (128, 128), (256, 128), (384, 57)]
GRAM_CHUNKS = [(0, 24), (24, 48), (48, 52)]  # ysB row ranges per PSUM bank

F32 = mybir.dt.float32
BF16 = mybir.dt.bfloat16


def build(reps=1, mm_only=False):
    """mm_only: skip bounce/transpose/scatter stages (HW experiment that
    isolates the matmul+eviction pipeline rate)."""
    nc = bacc.Bacc("TRN2", target_bir_lowering=False, debug=False, num_devices=8)
    in1p = nc.declare_dram_parameter("in1p", [2, 128, 4, NXS, NYS], BF16, isOutput=False)
    in2p = nc.declare_dram_parameter("in2p", [4, 128, 2, RB, CB], BF16, isOutput=False)
    outp = nc.declare_dram_parameter("out", [ND, H, W], F32, isOutput=True)

    with tile.TileContext(nc) as tc:
        with ExitStack() as ctx:
            const_pool = ctx.enter_context(tc.tile_pool(name="const", bufs=1))
            in2_pool = ctx.enter_context(tc.tile_pool(name="in2", bufs=2))
            band_pool = ctx.enter_context(tc.tile_pool(name="band", bufs=2))
            dense_pool = ctx.enter_context(tc.tile_pool(name="dense", bufs=2))
            out_pool = ctx.enter_context(tc.tile_pool(name="outsb", bufs=1))
            hbm_pool = ctx.enter_context(tc.tile_pool(name="hbm", bufs=2, space="DRAM"))
            pg_pool = ctx.enter_context(tc.tile_pool(name="pg", bufs=2, space="PSUM"))
            pt_pool = ctx.enter_context(tc.tile_pool(name="pt", bufs=2, space="PSUM"))

            ident = const_pool.tile([128, 128], BF16)
            make_identity(nc, ident)

            # resident in1: [c, k, cls, xs, ys]
            in1_sb = const_pool.tile([128, 2, 4, NXS, NYS], BF16)
            nc.sync.dma_start(
                out=bass.AP(in1_sb.tensor, in1_sb.offset,
                            [[2 * 4 * NXS * NYS, 128], [4 * NXS * NYS, 2], [1, 4 * NXS * NYS]]),
                in_=bass.AP(in1p, 0,
                            [[4 * NXS * NYS, 128], [128 * 4 * NXS * NYS, 2], [1, 4 * NXS * NYS]]),
            )

            # persistent d-major assembly buffers, one per d-chunk
            out_sb = [out_pool.tile([128, NPIX], BF16, tag=f"out{dc}", name=f"out_sb{dc}")
                      for dc in range(4)]

            def transpose_scatter(cid, dense, eng_flip):
                """PE transposes dense [pixel, d] to [d, pixel]; DVE/ACT
                scatter into the d-major raster assembly buffers."""
                py, px = cid // 2, cid % 2
                for dc, (d0, dcw) in enumerate(DCHUNKS):
                    for s in range(4):
                        pt = pt_pool.tile([128, 384], BF16)
                        for j in range(3):
                            nc.tensor.transpose(
                                pt[0:dcw, j * 128:(j + 1) * 128],
                                dense[:, 3 * s + j, d0:d0 + dcw],
                                ident[:],
                            )
                        ob = out_sb[dc]
                        src = bass.AP(pt.tensor, pt.offset,
                                      [[384, dcw], [128, 3], [32, 4], [1, 32]])
                        doff = 96 * py + px + 8 * (3 * s)
                        dst = bass.AP(ob.tensor, ob.offset + doff,
                                      [[NPIX, dcw], [8, 3], [2, 4], [192, 32]])
                        if eng_flip % 2 == 0:
                            nc.vector.tensor_copy(out=dst, in_=src)
                        else:
                            nc.scalar.copy(out=dst, in_=src)
                        eng_flip += 1
                return eng_flip

            eng_flip = 0
            pending = None  # (cid, dense) whose transpose stage is deferred
            for rep in range(reps):
              for cid in range(4):
                # one contiguous-per-partition load: [c, k, row, col]
                in2_sb = in2_pool.tile([128, 2, RB, CB], BF16)
                nc.scalar.dma_start(
                    out=bass.AP(in2_sb.tensor, in2_sb.offset,
                                [[2 * RB * CB, 128], [1, 2 * RB * CB]]),
                    in_=bass.AP(in2p, cid * 128 * 2 * RB * CB,
                                [[2 * RB * CB, 128], [1, 2 * RB * CB]]),
                )
                band = band_pool.tile([128, FB], BF16)
                evs = []
                for xsg in range(12):
                    pg = pg_pool.tile([128, 3, 512], F32)
                    # xg innermost: consecutive matmuls target different PE
                    # column tiles, so their moving streams overlap in the
                    # array (same-tile chunks would serialize).
                    for k in range(2):
                        for ch, (r0, r1) in enumerate(GRAM_CHUNKS):
                            ncols = (r1 - r0) * 21
                            for xg in range(4):
                                x0 = 4 * xsg + xg
                                lhsT = in1_sb[:, k, cid, x0, :]
                                rhs = in2_sb[:, k, r0:r1, x0:x0 + 21]
                                nc.tensor.matmul(
                                    pg[32 * xg:32 * (xg + 1), ch, 0:ncols],
                                    lhsT, rhs,
                                    start=(k == 0), stop=(k == 1),
                                    tile_position=(0, 32 * xg),
                                    skip_group_check=True,
                                )
                    # evict psum band into packed band columns; big chunk
                    # (banks 0-1, 1008 cols) and small chunk (bank 2, 84 cols)
                    # on opposite engines, alternating per xsg for balance.
                    big_src = bass.AP(pg.tensor, pg.offset, [[1536, 128], [512, 2], [1, 504]])
                    big_dst = bass.AP(band.tensor, band.offset + xsg * WB,
                                      [[FB, 128], [504, 2], [1, 504]])
                    small_src = pg[:, 2, 0:84]
                    small_dst = band[:, xsg * WB + 1008: xsg * WB + 1092]
                    if xsg % 2 == 0:
                        evs.append(nc.vector.tensor_copy(out=big_dst, in_=big_src))
                        evs.append(nc.scalar.copy(out=small_dst, in_=small_src))
                    else:
                        evs.append(nc.scalar.copy(out=big_dst, in_=big_src))
                        evs.append(nc.vector.tensor_copy(out=small_dst, in_=small_src))

                if mm_only:
                    continue
                # de-shear via an HBM bounce: the write is a linear dump
                # (partition-dim first so the cost model prices it right);
                # the 4 per-xg read-backs absorb the 21*ys shear on the HBM
                # side, where strides are unconstrained.
                hb = hbm_pool.tile([128, FB], BF16)
                wr = nc.sync.dma_start(
                    out=bass.AP(hb.tensor, hb.offset, [[FB, 128], [1, FB]]),
                    in_=bass.AP(band.tensor, band.offset, [[FB, 128], [1, FB]]),
                )
                for ev in evs:
                    tile.add_dep_helper(wr.ins, ev.ins,
                                        reason="hb write needs all evictions")
                dense = dense_pool.tile([128, NG, ND], BF16)
                for xg in range(4):
                    src = bass.AP(hb.tensor, hb.offset + 32 * xg * FB,
                                  [[FB + 21, 32], [WB, NG], [1, ND]])
                    dst = bass.AP(dense.tensor, dense.offset + 32 * xg * DP,
                                  [[DP, 32], [ND, NG], [1, ND]])
                    eng = nc.scalar if xg % 2 == 0 else nc.sync
                    eng.dma_start(out=dst, in_=src)

                # transpose/scatter the PREVIOUS class so PE's in-order
                # stream never stalls on this class's bounce DMAs.
                if pending is not None:
                    eng_flip = transpose_scatter(*pending, eng_flip)
                pending = (cid, dense)

              if pending is not None:
                  eng_flip = transpose_scatter(*pending, eng_flip)
                  pending = None

              # output: one cast DMA per d-chunk, 24KB contiguous runs per d
              for dc, (d0, dcw) in enumerate(DCHUNKS):
                  ob = out_sb[dc]
                  nc.gpsimd.dma_start(
                      out=bass.AP(outp, d0 * NPIX, [[NPIX, dcw], [1, NPIX]]),
                      in_=bass.AP(ob.tensor, ob.offset, [[NPIX, dcw], [1, NPIX]]),
                  )

    nc.compile()
    return nc


def prep_inputs(input1, input2):
    """Host-side: parity split, pad, bf16 cast, fold 1/256 into in1."""
    in_maps = []
    for b in range(B):
        a1 = (input1[b].astype(np.float32) / 256.0).reshape(2, 128, H, W)
        a2 = input2[b].astype(np.float32).reshape(2, 128, H, W)
        in1p = np.empty((2, 128, 4, NXS, NYS), dtype=ml_dtypes.bfloat16)
        in2p = np.zeros((4, 128, 2, RB, CB), dtype=ml_dtypes.bfloat16)
        for cid in range(4):
            py, px = cid // 2, cid % 2
            in1p[:, :, cid] = a1[:, :, py::2, px::2].transpose(0, 1, 3, 2).astype(ml_dtypes.bfloat16)
            in2p[cid, :, :, 10:42, 10:58] = a2[:, :, py::2, px::2].transpose(1, 0, 2, 3).astype(ml_dtypes.bfloat16)
        in_maps.append({"in1p": in1p, "in2p": in2p})
    return in_maps


_NC = None


def get_nc():
    global _NC
    if _NC is None:
        _NC = build()
    return _NC


def kernel(input1, input2):
    nc = get_nc()
    in_maps = prep_inputs(np.asarray(input1), np.asarray(input2))
    r = run_bass_kernel_spmd(nc, in_maps, core_ids=list(range(8)))
    return np.stack([r.results[i]["out"] for i in range(B)]).astype(np.float32)
